# revision 33
# baseline (speedup 1.0000x reference)
"""Trainium2 Bass kernel for CustomRationalLayer.

Math (B=256 batch, I=512 inputs, O=512 outputs):
    t = tanh(x * tanh_range)                                  (B, I)
    mole[b,o,i] = sum_{p=0..5} mc[o,i,p] t[b,i]^p
    deno[b,o,i] = sum_{q=1..4} dc[o,i,q-1] t[b,i]^q
    out[b,o]    = sum_i mole / (1 + |deno * x[b,i]|)

Strategy: tensor-parallel over O (64 outputs per core -> the large coef
tensors are split 8-way).  Per core:
  - power rows t^r and u_q = t^q * x computed once in (i-partition,
    b-free) layout as bf16, then round-tripped through DRAM so a single
    wide strided DMA per phase reloads them in matmul-moving layout.
    One COMBINED tile pw[20, PHJ, B] per phase: rows 0:2 ones (memset
    once into 4 persistent buffers; carries the constant mole coef via
    the block-diagonal weight rows), rows 2:12 = t^1..t^5 (pm moving =
    rows 0:12), rows 12:20 = u_1..u_4 (pd moving, stationary loaded at
    partition base 12, tile_position=(0,0)).  The single 18-partition
    reload is ~2x faster than separate 12p+8p reloads (DMA throughput
    is per-partition-limited, ~2.6 GB/s/partition/queue).
  - The dumps go out on the scalar (ACT) HW DGE queue one strip per
    phase, two phases ahead of use; reloads + weights on the sync (SP)
    queue.  Weights are host-packed phase-contiguous.
  - i is processed as 256 pairs j = (i, i+256), K=12 bf16 matmul for
    the numerator, K=8 for deno*x, per pair.  Elementwise on
    [128, 4, B] group tiles: abs (split ACT/DVE 5:3), ACT
    reciprocal(z+1), DVE ratio = pm * rcp as bf16.  GPSIMD tree-folds
    ratio tiles (pairs, quads); DVE folds quad-pairs into octo tiles
    one block late (so the strict-FIFO engine queues never head-of-line
    block on the fold chain), and one identity-matmul pair per octo
    accumulates the i-sum into PSUM.
Output per core is (64 o, 256 b); host transposes and concatenates.
"""

import numpy as np
import ml_dtypes

import concourse.bass as bass
import concourse.tile as tile
from concourse import bacc, mybir
from concourse.bass_utils import run_bass_kernel_spmd

B = 256
I = 512
O = 512
NC = 8
OSH = O // NC          # outputs per core
NJ = I // 2            # i-pairs per core
PHJ = 16               # pairs per staging phase
NPH = NJ // PHJ
F32 = mybir.dt.float32
BF16 = mybir.dt.bfloat16
ALU = mybir.AluOpType
AF = mybir.ActivationFunctionType

_CACHE = {}


def _act_reciprocal(nc, out, in_, bias):
    """ACT Reciprocal via raw InstActivation (the bass wrapper bans it; the
    measured accuracy of reciprocal(x+bias) on TRN2 is ~1.2e-5 max rel err,
    well inside this kernel's bf16 noise floor)."""
    eng = nc.scalar
    ins = [eng.lower_ap(in_)]
    for val in (float(bias), 1.0, 0.0):  # bias, scale, alpha
        ins.append(mybir.ImmediateValue(dtype=mybir.dt.float32, value=val))
    return eng.add_instruction(mybir.InstActivation(
        name=nc.get_next_instruction_name(),
        func=AF.Reciprocal,
        ins=ins,
        outs=[eng.lower_ap(out)],
    ))


def _build_bass():
    nc = bacc.Bacc("TRN2", target_bir_lowering=False, debug=False, num_devices=NC)

    XP = nc.dram_tensor("xp", [128, 2, 2, B], BF16, kind="ExternalInput").ap()
    TRB = nc.dram_tensor("trb", [128, 1], F32, kind="ExternalInput").ap()
    WM = nc.dram_tensor("wm", [NPH, 12, PHJ, 128], BF16, kind="ExternalInput").ap()
    WD = nc.dram_tensor("wd", [NPH, 8, PHJ, 128], BF16, kind="ExternalInput").ap()
    ID2 = nc.dram_tensor("id2", [128, OSH], BF16, kind="ExternalInput").ap()
    OUT = nc.dram_tensor("out_y", [OSH, B], F32, kind="ExternalOutput").ap()

    with tile.TileContext(nc) as tc:
        with (
            tc.tile_pool(name="consts", bufs=1) as consts,
            tc.tile_pool(name="powers", bufs=1) as powers,
            tc.tile_pool(name="dramp", bufs=1, space="DRAM") as dramp,
            tc.tile_pool(name="wmp", bufs=4) as wmp,
            tc.tile_pool(name="wdp", bufs=1) as wdp,
            tc.tile_pool(name="work", bufs=4) as work,
            tc.tile_pool(name="rrp", bufs=6) as rrp,
            tc.tile_pool(name="ssp", bufs=4) as ssp,
            tc.tile_pool(name="qp", bufs=4) as qp,
            tc.tile_pool(name="oqp", bufs=3) as oqp,
            tc.tile_pool(name="outp", bufs=1) as outp,
            tc.tile_pool(name="pmp", bufs=2, space="PSUM") as pmp,
            tc.tile_pool(name="pdp", bufs=3, space="PSUM") as pdp,
            tc.tile_pool(name="accp", bufs=1, space="PSUM") as accp,
        ):
            # x first on the SP queue: it heads the compute critical path.
            # Host-prepacked to (p, c1, par, b) so each c1-half is a
            # contiguous 1KB-per-partition transfer; the c1=0 half lands
            # first and the power chain starts on it while c1=1 is still
            # in flight.  i = 256*par + 128*c1 + p
            XB = powers.tile([128, 2, 2, B], BF16)
            nc.sync.dma_start(out=XB[:, 0], in_=XP[:, 0])
            trb_s = consts.tile([128, 1], F32)
            nc.sync.dma_start(out=trb_s, in_=TRB)
            id2_s = consts.tile([128, OSH], BF16)
            nc.sync.dma_start(out=id2_s, in_=ID2)
            nc.sync.dma_start(out=XB[:, 1], in_=XP[:, 1])

            # preload the Tanh activation table while the x DMA is in flight
            warm = consts.tile([1, 1], F32)
            nc.gpsimd.memset(warm, 0.0)
            warm2 = consts.tile([1, 1], BF16)
            nc.scalar.activation(warm2, warm, AF.Tanh)

            # 4 persistent phase-powers tiles; rows 0:2 are the ones rows
            # (constant-coef carrier), written once (below), never re-DMAed.
            pw_bufs = [
                powers.tile([20, PHJ, B], BF16, tag=f"pw{i}", name=f"pw{i}")
                for i in range(4)
            ]

            # phase-weight staging, prefetched 3 phases deep.  The deno
            # stationary is K=20 with rows 0:12 zero (matmul operands must
            # start at partition 0/32/64/96, and cycle cost is column-
            # count-bound, so the pad rows are free); zeros are memset once
            # into 4 persistent buffers, the DMA only fills rows 12:20.
            wd_bufs = [
                wdp.tile([20, PHJ, 128], BF16, tag=f"wdb{i}", name=f"wdb{i}")
                for i in range(4)
            ]
            for i, t_ in enumerate(wd_bufs):
                nc.gpsimd.memset(t_[0:12], 0.0)
                nc.gpsimd.memset(pw_bufs[i][0:2], 1.0)
            wtiles = {}

            def stage_weights(ph):
                wm_t = wmp.tile([12, PHJ, 128], BF16)
                nc.sync.dma_start(out=wm_t, in_=WM[ph])
                wd_t = wd_bufs[ph % 4]
                nc.sync.dma_start(out=wd_t[12:20], in_=WD[ph])
                wtiles[ph] = (wm_t, wd_t)

            stage_weights(0)

            # Power/u rows in (i-partition, b-free) layout, one combined
            # tile so each strip dumps with a single DMA.  Slot s=0..4 is
            # t^(s+1); slot 5..8 is u_q = t^q * x (q = s-4).  Computed one
            # c1-half at a time so the c1=0 strips can dump (and the first
            # phases' reloads start) while the c1=1 half is still being
            # computed.  All muls on DVE (GPSIMD sharing these SBUF tiles
            # was measured to stretch both engines ~2x).
            Xv = XB
            PW = powers.tile([128, 2, 9, 2, B], BF16)
            DTs = {}

            # HAM warm-up: ~4.5us of dense K=128 matmul activity while the
            # PE would otherwise sit idle waiting for the first powers
            # reload.  The PE clock gate defaults to 4/8 (1.2 GHz) and only
            # releases to 2.4 GHz after a full ~3.4us busy window; the
            # K=12/20 production matmuls appear not to trigger it by
            # themselves.  Results land in a scratch PSUM tile and are
            # never read.
            warm_ps = pmp.tile([128, 4, B], F32, name="warm_ps", tag="pm")
            for w in range(24):
                nc.tensor.matmul(
                    warm_ps[0:OSH, 2 * (w % 2) : 2 * (w % 2) + 2],
                    id2_s, XB[:, w % 2].rearrange("p par b -> p (par b)"),
                    start=True, stop=True,
                )

            def dump_strip(ph):
                s, c = ph % 8, ph // 8
                dt = dramp.tile([16, 9, 2, B], BF16, tag=f"dt{ph}")
                nc.scalar.dma_start(out=dt, in_=PW[16 * s : 16 * (s + 1), c])
                DTs[ph] = dt.rearrange("p s par b -> s par p b")

            ptiles = {}

            def stage_powers(ph):
                with tc.high_priority(offset=400):
                    pw = pw_bufs[ph % 4]
                    nc.sync.dma_start(out=pw[2:20], in_=DTs[ph])
                ptiles[ph] = pw

            def powers_half(c):
                T, Xc = PW[:, c], Xv[:, c]
                nc.scalar.activation(T[:, 0], Xc, AF.Tanh, scale=trb_s[:, 0:1])
                nc.vector.tensor_mul(T[:, 1], T[:, 0], T[:, 0])
                nc.vector.tensor_mul(T[:, 5], T[:, 0], Xc)
                nc.vector.tensor_mul(T[:, 2], T[:, 1], T[:, 0])
                nc.vector.tensor_mul(T[:, 6], T[:, 1], Xc)
                nc.vector.tensor_mul(T[:, 3], T[:, 1], T[:, 1])
                nc.vector.tensor_mul(T[:, 7], T[:, 2], Xc)
                nc.vector.tensor_mul(T[:, 8], T[:, 3], Xc)
                nc.vector.tensor_mul(T[:, 4], T[:, 3], T[:, 0])

            powers_half(0)
            dump_strip(0)
            dump_strip(1)
            stage_powers(0)
            stage_weights(1)
            stage_powers(1)
            stage_weights(2)
            powers_half(1)

            # [64, 2, B]: the N=512 identity matmul leaves the two packed
            # pairs side by side; folded after the loop.
            acc = accp.tile([OSH, 2, B], F32)

            # 2 identity matmuls per octo for blocks 0..6, 8 direct-ss
            # identity matmuls for the last block (its folds run on DVE and
            # feed the PE directly -- the GPSIMD fold-chain latency would
            # otherwise be exposed as a serial tail)
            NQ = 2 * (NJ // 32 - 1) + 8
            opending = []
            n_ident = 0

            def emit_ident(mv):
                nonlocal n_ident
                nc.tensor.matmul(
                    acc, id2_s, mv,
                    start=(n_ident == 0), stop=(n_ident == NQ - 1),
                )
                n_ident += 1

            def flush_ident(limit):
                while len(opending) > limit:
                    q = opending.pop(0)
                    emit_ident(q[:, 0:2])
                    emit_ident(q[:, 2:4])

            # octo folds run one 8-group block late so the DVE add never
            # waits at the head of the FIFO for the GPSIMD quad folds
            qq_ready = []
            ss_pending = []

            def do_octo():
                qq = qq_ready.pop(0)
                oq = oqp.tile([128, 4, B], BF16)
                nc.vector.tensor_add(oq, qq[:, 0:4], qq[:, 4:8])
                opending.append(oq)
                flush_ident(1)

            for ph in range(NPH):
                if ph + 2 < NPH:
                    dump_strip(ph + 2)
                    stage_powers(ph + 2)
                if ph + 3 < NPH:
                    stage_weights(ph + 3)
                wm_s, wd_s = wtiles.pop(ph)
                pw = ptiles.pop(ph)

                for g4 in range(PHJ // 4):   # four pairs per elementwise group
                    gidx = (PHJ // 4) * ph + g4
                    last_blk = gidx >= NJ // 4 - 8
                    # deno PSUM tiles are b-split (1 bank each) so a single
                    # 2-bank pm tile fits: one ratio-mul DVE instruction per
                    # group and a reciprocal batched over two groups
                    pm4 = pmp.tile([128, 4, B], F32, tag="pm")
                    pdh = [
                        pdp.tile([128, 4, B // 2], F32, tag="pd", name=f"pd{bh}")
                        for bh in range(2)
                    ]
                    with tc.high_priority(offset=80):
                        for bh in range(2):
                            bs = (B // 2) * bh
                            for k in range(4):
                                jl = 4 * g4 + k
                                nc.tensor.matmul(
                                    pdh[bh][:, k],
                                    wd_s[:, jl, :],
                                    pw[:, jl, bs : bs + B // 2],
                                    start=True, stop=True,
                                )
                        for k in range(4):
                            jl = 4 * g4 + k
                            nc.tensor.matmul(
                                pm4[:, k],
                                wm_s[:, jl, :], pw[0:12, jl, :],
                                start=True, stop=True,
                            )
                    if gidx % 2 == 0:
                        z8 = work.tile([128, 2, 4, B], BF16, tag="z")
                    zh = z8[:, gidx % 2]
                    for bh in range(2):
                        bs = (B // 2) * bh
                        if gidx % 10 < 3:
                            # DVE abs: reduce over a trailing unit axis with
                            # the |.| modifier (offloads ACT)
                            nc.vector.tensor_reduce(
                                zh[:, :, bs : bs + B // 2],
                                pdh[bh].rearrange("p a (b one) -> p a b one", one=1),
                                mybir.AxisListType.X,
                                ALU.max,
                                apply_absolute_value=True,
                            )
                        else:
                            nc.scalar.activation(
                                zh[:, :, bs : bs + B // 2], pdh[bh], AF.Abs
                            )
                    if gidx % 2 == 1:
                        rcp8 = work.tile([128, 2, 4, B], BF16, tag="rcp")
                        # ACT path: reciprocal(z + 1) over two groups at
                        # once (the 352-cycle ACT ramp amortizes); abs and
                        # reciprocal share one table set -> no reloads
                        _act_reciprocal(nc, rcp8, z8, 1.0)
                    # ratios of two consecutive groups share one [128,8,B]
                    # tile so each GPSIMD fold level is a single big op
                    if gidx % 2 == 0:
                        rr = rrp.tile([128, 8, B], BF16)
                    if gidx % 4 == 0:
                        ss = ssp.tile([128, 8, B], BF16)
                    if gidx % 2 == 1:
                        nc.vector.tensor_mul(
                            rr[:, 0:4], pm_prev, rcp8[:, 0]
                        )
                        nc.vector.tensor_mul(
                            rr[:, 4:8], pm4, rcp8[:, 1]
                        )
                    pm_prev = pm4
                    if gidx % 2 == 1:
                        sh = 4 * ((gidx // 2) % 2)
                        if last_blk:
                            # final block: folds on DVE, identity straight
                            # from ss -- no GPSIMD latency in the tail
                            nc.vector.tensor_add(
                                ss[:, sh : sh + 4], rr[:, 0:4], rr[:, 4:8]
                            )
                        else:
                            nc.gpsimd.tensor_add(
                                ss[:, sh : sh + 4], rr[:, 0:4], rr[:, 4:8]
                            )
                    if gidx % 4 == 3:
                        if last_blk:
                            while len(qq_ready) > 1:
                                do_octo()
                            ss_pending.append(ss)
                            if len(ss_pending) > 1:
                                s0 = ss_pending.pop(0)
                                for k in range(0, 8, 2):
                                    emit_ident(s0[:, k : k + 2])
                        else:
                            qh = 4 * ((gidx // 4) % 2)
                            if gidx % 8 == 3:
                                qq = qp.tile([128, 8, B], BF16)
                                # octo folds run TWO blocks late: GPSIMD's
                                # queue lags ~a phase, one block of slack
                                # was not enough
                                if len(qq_ready) > 1:
                                    do_octo()
                            nc.gpsimd.tensor_add(
                                qq[:, qh : qh + 4], ss[:, 0:4], ss[:, 4:8]
                            )
                            if gidx % 8 == 7:
                                qq_ready.append(qq)

            while qq_ready:
                do_octo()
            for s0 in ss_pending:
                for k in range(0, 8, 2):
                    emit_ident(s0[:, k : k + 2])
            flush_ident(0)

            acc_s = outp.tile([OSH, 2, B], F32)
            nc.scalar.copy(acc_s, acc)
            out_s = outp.tile([OSH, B], F32)
            nc.vector.tensor_add(out_s, acc_s[:, 0], acc_s[:, 1])
            nc.sync.dma_start(out=OUT, in_=out_s)

    nc.compile()
    return nc


def _prep_inputs(x, tanh_range, mole_coef, deno_coef):
    """Host-side prepack -> list of per-core input maps.

    W row order for the pair j=(i, i+256): row 2r+par = coef of power r
    for i + 256*par; columns 0:64 hold par=0 outputs, 64:128 par=1 outputs.
    Row pair 0/1 (power 0, the ones row) carries the constant coef mc0.
    Weights are phase-major so each phase's block is one contiguous DMA.
    """
    bf16 = ml_dtypes.bfloat16
    xt = np.ascontiguousarray(x.T).astype(bf16)          # (I, B)
    xp = np.ascontiguousarray(
        xt.reshape(2, 2, 128, B).transpose(2, 1, 0, 3)   # (p, c1, par, b)
    )
    trb = np.full((128, 1), np.float32(tanh_range), dtype=np.float32)
    id2 = np.concatenate([np.eye(OSH), np.eye(OSH)], axis=0).astype(bf16)
    in_maps = []
    for c in range(NC):
        o0 = OSH * c
        mc = mole_coef[o0 : o0 + OSH]  # (64, 512, 6)
        dc = deno_coef[o0 : o0 + OSH]  # (64, 512, 4)
        wm = np.zeros((12, NJ, 128), dtype=np.float32)
        wd = np.zeros((8, NJ, 128), dtype=np.float32)
        for r in range(6):
            wm[2 * r, :, 0:OSH] = mc[:, 0:NJ, r].T
            wm[2 * r + 1, :, OSH:128] = mc[:, NJ:I, r].T
        for r in range(4):
            wd[2 * r, :, 0:OSH] = dc[:, 0:NJ, r].T
            wd[2 * r + 1, :, OSH:128] = dc[:, NJ:I, r].T
        wmp = np.ascontiguousarray(
            wm.reshape(12, NPH, PHJ, 128).transpose(1, 0, 2, 3)
        ).astype(bf16)
        wdp = np.ascontiguousarray(
            wd.reshape(8, NPH, PHJ, 128).transpose(1, 0, 2, 3)
        ).astype(bf16)
        in_maps.append(
            {
                "xp": xp,
                "trb": trb,
                "wm": wmp,
                "wd": wdp,
                "id2": id2,
            }
        )
    return in_maps


def kernel(x, tanh_range, mole_coef, deno_coef):
    x = np.asarray(x, dtype=np.float32)
    mole_coef = np.asarray(mole_coef, dtype=np.float32)
    deno_coef = np.asarray(deno_coef, dtype=np.float32)
    if "nc" not in _CACHE:
        _CACHE["nc"] = _build_bass()
    nc = _CACHE["nc"]
    in_maps = _prep_inputs(x, tanh_range, mole_coef, deno_coef)
    res = run_bass_kernel_spmd(nc, in_maps, list(range(NC)))
    out = np.empty((B, O), dtype=np.float32)
    for c in range(NC):
        out[:, OSH * c : OSH * (c + 1)] = res.results[c]["out_y"].T
    return out
